# revision 56
# baseline (speedup 1.0000x reference)
# Trainium2 Bass kernel for NormalAttention (1x1-conv q/k/v attention over HW).
#
# Math (per batch b):
#   q = Wq x + bq            [64, 4096]
#   k = Wk x + bk            [64, 4096]
#   v = Wv x + bv            [256, 4096]
#   E[i,j] = sum_c q[c,i] k[c,j]          (energy, [4096, 4096])
#   A = elu(E) / 4096
#   out = v @ A                           [256, 4096]
#   y = Wg out + bg
#
# Kernel strategy (VARIANT v5, default): data-parallel, one batch per
# NeuronCore (8 cores).  Per core, a flash-attention-style stream over
# the energy matrix; E is never materialized in HBM.
#
#   * The value and gamma convs are fused on the host: W2 = Wg Wv / N,
#     so the PE accumulates y' = (W2^T x)^T @ G directly -- no separate
#     gamma stage, one less PSUM round-trip.
#   * G = elu(E)+1 = min(exp(E),1) + relu(E) = min(exp(E), 1+relu(E)),
#     computed as ONE ACT op (t=exp(E), the only ACT op per pair -- ACT
#     ops measured ~2x their modeled cost, so ACT is kept to the bare
#     exp pass) plus two DVE ops: r=(E max 0)+1 (a two-scalar-op
#     tensor_scalar that is the LAST reader of the PSUM tile, so the
#     eps ring frees early and the in-order PE never stalls on E(j+3);
#     measured 29% faster than the variant whose final DVE op re-reads
#     PSUM) and G=min(r,t) (bf16 2x mode).  The +1 offset is exactly
#     corrected via the bias: y = y' + (bg - rowsum(W2^T x)).
#   * GPSIMD is deliberately unused: its SBUF port is an exclusive lock
#     shared with the DVE's second port, so Pool streaming ops
#     serialize against DVE and cost ~3x what the cost model claims.
#   * PSUM (v5d): 2 eps tiles (4 banks) + DOUBLE-buffered out
#     accumulators (4 banks).  With the early-free elementwise order the
#     3rd eps buffer is dead weight; spending those banks on the out
#     accumulators removes the per-m-tile stall where out(mt+1, p0)
#     waited on fo(mt) draining (11% faster head-to-head).  Out-matmuls
#     trail the E-matmuls by OUT_LAG=4 pairs.
import os

import numpy as np
import ml_dtypes

import concourse.bass as bass
import concourse.mybir as mybir
import concourse.tile as tile
from concourse import bacc
from concourse.bass_utils import run_bass_kernel_spmd

B, C, HH, WW = 8, 256, 64, 64
N = HH * WW          # 4096 spatial positions
CQ = 64              # query/key channels
NCORES = 8
MT = 512             # m (energy column) tile
NPAIRS = 16          # pairs of 128-row n-chunks per m-tile
ACT_RELU_EVERY = 3   # 1/ACT_RELU_EVERY of relu passes go to ACT (rest DVE)
DVE_COMBINE_UPTO = 9  # of every 16 combine ops, this many on DVE (rest Pool)
POOL_PAIRS = int(os.environ.get("POOL_PAIRS", "5"))   # of 16: combines on Pool
ACT_RELU32 = int(os.environ.get("ACT_RELU32", "0"))   # of 32: relus on ACT
OUT_LAG = int(os.environ.get("OUT_LAG", "4"))         # v3 pipeline depth
VT_ON_DVE = os.environ.get("VT_ON_DVE", "1") == "1"   # vt/qk copies on DVE
VARIANT = os.environ.get("KVAR", "v5d")  # v5d (default) | v5b | ...

F32 = mybir.dt.float32
F32R = mybir.dt.float32r
BF16 = mybir.dt.bfloat16
AL = mybir.AluOpType
AF = mybir.ActivationFunctionType


def build_nc(reps=1, variant=None):
    global VARIANT
    if variant is not None:
        VARIANT = variant
    QDT = BF16 if VARIANT == "bf16" else F32R
    V5 = VARIANT.startswith(("v5", "v6", "v7"))
    nc = bacc.Bacc("TRN2", target_bir_lowering=False, debug=False,
                   num_devices=NCORES)
    xd = nc.declare_dram_parameter("x", [2, 128, N], QDT, isOutput=False)
    wqd = nc.declare_dram_parameter("wqT", [2, 128, CQ], QDT, isOutput=False)
    wkd = nc.declare_dram_parameter("wkT", [2, 128, CQ], QDT, isOutput=False)
    bqd = nc.declare_dram_parameter("bq", [CQ, 1], F32, isOutput=False)
    bkd = nc.declare_dram_parameter("bk", [CQ, 1], F32, isOutput=False)
    if V5:
        # fused value+gamma conv: W2 = Wg Wv / N, b2 = Wg bv / N (host-side)
        wvd = nc.declare_dram_parameter("w2T", [2, 128, C], QDT,
                                        isOutput=False)
        bvd = nc.declare_dram_parameter("b2", [1, C], QDT, isOutput=False)
        wgd = None
    else:
        wvd = nc.declare_dram_parameter("wvT", [2, 128, C], QDT,
                                        isOutput=False)
        bvd = nc.declare_dram_parameter("bv", [1, C], QDT, isOutput=False)
        wgd = nc.declare_dram_parameter("wgT16", [2, 128, C], BF16,
                                        isOutput=False)
    bgd = nc.declare_dram_parameter("bg", [C, 1], F32, isOutput=False)
    onesd = nc.declare_dram_parameter("ones", [1, 128], QDT, isOutput=False)
    od = nc.declare_dram_parameter("out", [2, 128, N], F32, isOutput=True)

    with tile.TileContext(nc) as tc:
        V3 = VARIANT.startswith("v3") or V5
        V6 = VARIANT.startswith("v6")
        V7 = VARIANT.startswith("v7")
        with (
            tc.tile_pool(name="wts", bufs=1) as wts,
            tc.tile_pool(name="xs", bufs=1) as xs,
            tc.tile_pool(name="qk", bufs=(2 if V7 else 1)) as qkp,
            tc.tile_pool(name="vt", bufs=(2 if V7 else 1)) as vtp,
            tc.tile_pool(name="elem", bufs=(OUT_LAG + 1 if V3 else 3)) as elem,
            tc.tile_pool(name="finp", bufs=2) as finp,
            tc.tile_pool(name="pse",
                         bufs=(2 if VARIANT == "v5d" else 1 if V6
                               else 3 if V3 else 2),
                         space="PSUM") as pse,
            tc.tile_pool(name="pso", bufs=(2 if VARIANT == "v5d" else 1),
                         space="PSUM") as pso,
            # unused in V3 (PSUM fully claimed by pse=3x2 + pso=2 banks)
            tc.tile_pool(name="psg", bufs=2, space="PSUM") as psg,
        ):
            def body(iv=None):
                # v6: one manually-rotated PSUM ring (3 slots x [128,1024]);
                # slots (0,1) host "paired" exp ops spanning two pairs.
                ring = pse.tile([128, 3, 2 * MT], F32, tag="eps",
                                name="ring") if V6 else None
                x_sb = [xs.tile([128, N], QDT, tag=f"x{i}", name=f"x_sb{i}")
                        for i in range(2)]
                for i in range(2):
                    for cch in range(4):
                        cs = slice(cch * (N // 4), (cch + 1) * (N // 4))
                        nc.sync.dma_start(x_sb[i][:, cs], xd[i][:, cs])
                wq_sb = wts.tile([128, 2, CQ], QDT, tag="wq", name="wq_sb")
                wk_sb = wts.tile([128, 2, CQ], QDT, tag="wk", name="wk_sb")
                wv_sb = wts.tile([128, 2, C], QDT, tag="wv", name="wv_sb")
                wg_sb = None if V5 else wts.tile([128, 2, C], BF16, tag="wg",
                                                name="wg_sb")
                for i in range(2):
                    nc.sync.dma_start(wq_sb[:, i, :], wqd[i])
                    nc.sync.dma_start(wk_sb[:, i, :], wkd[i])
                    nc.sync.dma_start(wv_sb[:, i, :], wvd[i])
                    if not V5:
                        nc.sync.dma_start(wg_sb[:, i, :], wgd[i])
                bq_sb = wts.tile([CQ, 1], F32, tag="bq", name="bq_sb")
                nc.sync.dma_start(bq_sb, bqd[:])
                bk_sb = wts.tile([CQ, 1], F32, tag="bk", name="bk_sb")
                nc.sync.dma_start(bk_sb, bkd[:])
                bv_sb = wts.tile([1, C], QDT, tag="bv", name="bv_sb")
                nc.sync.dma_start(bv_sb, bvd[:])
                bg_sb = wts.tile([128, 2], F32, tag="bg", name="bg_sb")
                for h in range(2):
                    nc.sync.dma_start(bg_sb[:, h:h + 1],
                                      bgd[h * 128:(h + 1) * 128, :])
                ones_row = wts.tile([1, 128], QDT, tag="ones_row",
                                    name="ones_row")
                nc.sync.dma_start(ones_row, onesd[:])
                ones_col = wts.tile([128, 1], BF16, tag="ones_col",
                                    name="ones_col")
                nc.vector.memset(ones_col, 1.0)

                q_sb = qkp.tile([2 * CQ, N], QDT, tag="q", name="q_sb")
                k_sb = qkp.tile([2 * CQ, N], QDT, tag="k", name="k_sb")
                vt_sb = vtp.tile([128, 32, C], BF16, tag="vt", name="vt_sb")
                # per-iteration scalars: double-buffered pool under V7 so
                # the next body's S-chain doesn't wait on this body's tail
                scp = finp if V7 else wts
                sT_sb = scp.tile([1, C], F32, tag="sT", name="sT_sb")
                s_col = scp.tile([128, 2], F32 if V5 else BF16, tag="scol",
                                 name="s_col")
                bge_sb = scp.tile([128, 2], F32, tag="bge", name="bge_sb")

                # ---- q, k = conv1x1(x) + bias   [64, 4096] ----
                if V3:
                    qk_slot = [0]
                    for ti in range(0, N // 512, 2):
                        for dst, w_s, b_s in ((q_sb, wq_sb, bq_sb),
                                              (k_sb, wk_sb, bk_sb)):
                            if V6:
                                psf = ring[:, qk_slot[0] % 3, :]
                                qk_slot[0] += 1
                            else:
                                psf = pse.tile([128, 2 * MT], F32, tag="eps",
                                               name="qkps")
                            for s in range(2):
                                sl = slice((ti + s) * 512, (ti + s + 1) * 512)
                                csl = slice(s * 512, (s + 1) * 512)
                                nc.tensor.matmul(psf[:CQ, csl], w_s[:, 0, :],
                                                 x_sb[0][:, sl],
                                                 start=True, stop=False)
                                nc.tensor.matmul(psf[:CQ, csl], w_s[:, 1, :],
                                                 x_sb[1][:, sl],
                                                 start=False, stop=True)
                            if VT_ON_DVE:
                                nc.vector.tensor_scalar(
                                    dst[:CQ, ti * 512:(ti + 2) * 512],
                                    psf[:CQ, :], b_s, None, AL.add)
                            else:
                                nc.scalar.activation(
                                    dst[:CQ, ti * 512:(ti + 2) * 512],
                                    psf[:CQ, :], AF.Identity, bias=b_s,
                                    scale=1.0)
                else:
                    for ti in range(N // 512):
                        sl = slice(ti * 512, (ti + 1) * 512)
                        for dst, w_s, b_s in ((q_sb, wq_sb, bq_sb),
                                              (k_sb, wk_sb, bk_sb)):
                            ps = psg.tile([CQ, 512], F32, tag="gps",
                                          name="qkps")
                            nc.tensor.matmul(ps, w_s[:, 0, :], x_sb[0][:, sl],
                                             start=True, stop=False)
                            nc.tensor.matmul(ps, w_s[:, 1, :], x_sb[1][:, sl],
                                             start=False, stop=True)
                            nc.scalar.activation(dst[:CQ, sl], ps,
                                                 AF.Identity,
                                                 bias=b_s, scale=1.0)
                # duplicate q/k into partitions 64..127 (PE row-group packing)
                for dst in (q_sb, k_sb):
                    nc.sync.dma_start(dst[CQ:2 * CQ, :], dst[:CQ, :])
                # ---- v^T = (x^T WvT + bv)/4096, stored bf16 [4096, 256] ----
                if V3:
                    # two n-chunks per PSUM tile -> wider (cheaper) ACT copies
                    for nj in range(16):
                        if V6:
                            ps = ring[:, nj % 3, 0:2 * C]
                        else:
                            ps = pse.tile([128, 2, C], F32, tag="eps",
                                          name="vps")
                        for s in range(2):
                            ni = 2 * nj + s
                            nsl = slice(ni * 128, (ni + 1) * 128)
                            pss = (ps[:, s * C:(s + 1) * C] if V6
                                   else ps[:, s, :])
                            nc.tensor.matmul(pss, x_sb[0][:, nsl],
                                             wv_sb[:, 0, :],
                                             start=True, stop=False)
                            nc.tensor.matmul(pss, x_sb[1][:, nsl],
                                             wv_sb[:, 1, :],
                                             start=False, stop=False)
                            nc.tensor.matmul(pss, ones_row, bv_sb,
                                             start=False, stop=True)
                        if VT_ON_DVE:
                            nc.vector.tensor_copy(
                                vt_sb[:, 2 * nj:2 * nj + 2, :], ps)
                        else:
                            nc.scalar.activation(
                                vt_sb[:, 2 * nj:2 * nj + 2, :], ps, AF.Copy)
                else:
                    for ni in range(32):
                        nsl = slice(ni * 128, (ni + 1) * 128)
                        ps = pse.tile([128, C], F32, tag="eps", name="vps")
                        nc.tensor.matmul(ps, x_sb[0][:, nsl], wv_sb[:, 0, :],
                                         start=True, stop=False)
                        nc.tensor.matmul(ps, x_sb[1][:, nsl], wv_sb[:, 1, :],
                                         start=False, stop=False)
                        nc.tensor.matmul(ps, ones_row, bv_sb,
                                         start=False, stop=True)
                        if VARIANT in ("cfga", "cfgb"):
                            nc.scalar.activation(vt_sb[:, ni, :], ps, AF.Copy)
                        else:
                            nc.vector.tensor_copy(vt_sb[:, ni, :], ps)
                # ---- S = rowsum(vt'); bg_eff = bg - Wg S (v5: bg - S') ----
                sps = pso.tile([1, C], F32, tag="o0", name="sps")
                for ni in range(32):
                    nc.tensor.matmul(sps, ones_col, vt_sb[:, ni, :],
                                     start=(ni == 0), stop=(ni == 31))
                nc.vector.tensor_copy(sT_sb, sps)
                for h in range(2):
                    # [1,128] row -> [128,1] column (with dtype cast)
                    nc.gpsimd.dma_start(s_col[:, h:h + 1],
                                        sT_sb[:, h * 128:(h + 1) * 128])
                if V5:
                    # vt' already includes Wg: bg_eff = bg - S'
                    nc.vector.tensor_tensor(bge_sb, bg_sb, s_col, AL.subtract)
                else:
                    for h in range(2):
                        hsl = slice(h * 128, (h + 1) * 128)
                        ps = pso.tile([128, 1], F32, tag="o1", name="bgps")
                        nc.tensor.matmul(ps, wg_sb[:, 0, hsl], s_col[:, 0:1],
                                         start=True, stop=False)
                        nc.tensor.matmul(ps, wg_sb[:, 1, hsl], s_col[:, 1:2],
                                         start=False, stop=True)
                        nc.scalar.activation(bge_sb[:, h:h + 1], ps,
                                             AF.Identity,
                                             bias=bg_sb[:, h:h + 1],
                                             scale=-1.0)

                # ---- main attention loop (software-pipelined) ----
                # PE stream: E(0), E(1), out(0), E(2), out(1), ... so the PE
                # never sits behind the elementwise chain of the current pair.
                pairs = [(mt, p) for mt in range(N // MT)
                         for p in range(NPAIRS)]
                g_q = {}
                o_ps = {}

                def emit_dve_chain(j, eps, t16):
                    # G = min(exp(E),1)+relu(E); all-DVE after exp
                    g16 = elem.tile([128, 2 * MT], BF16, tag="g", name="g16")
                    if VARIANT == "v6b":
                        # early-eps-free order (v5b-style)
                        r16 = elem.tile([128, 2 * MT], BF16, tag="r",
                                        name="r16")
                        nc.vector.tensor_scalar(r16, eps, 0.0, 1.0,
                                                AL.max, AL.add)
                        nc.vector.tensor_tensor(g16, r16, t16, AL.min)
                        return g16
                    m16 = elem.tile([128, 2 * MT], BF16, tag="m", name="m16")
                    nc.vector.tensor_scalar(m16, t16, 1.0, None, AL.min)
                    nc.vector.scalar_tensor_tensor(
                        g16, eps, 0.0, m16, AL.max, AL.add)
                    return g16

                def emit_e6(j):
                    mt, p = pairs[j]
                    msl = slice(mt * MT, (mt + 1) * MT)
                    nA, nB = 2 * p, 2 * p + 1
                    eps = ring[:, j % 3, :]
                    nc.tensor.matmul(eps[:, 0:MT],
                                     q_sb[:CQ, nA * 128:(nA + 1) * 128],
                                     k_sb[:CQ, msl], start=True, stop=True)
                    nc.tensor.matmul(eps[:, MT:2 * MT],
                                     q_sb[CQ:2 * CQ, nB * 128:(nB + 1) * 128],
                                     k_sb[CQ:2 * CQ, msl],
                                     start=True, stop=True)
                    # exp + DVE chain, batched 2 pairs per ACT op on slots 0,1
                    if j % 3 == 1:
                        t2 = elem.tile([128, 2, 2 * MT], BF16, tag="t2",
                                       name="t2")
                        nc.scalar.activation(t2, ring[:, 0:2, :], AF.Exp)
                        g_q[j - 1] = emit_dve_chain(j - 1, ring[:, 0, :],
                                                    t2[:, 0, :])
                        g_q[j] = emit_dve_chain(j, ring[:, 1, :],
                                                t2[:, 1, :])
                    elif j % 3 == 2:
                        t16 = elem.tile([128, 2 * MT], BF16, tag="t",
                                        name="t16")
                        nc.scalar.activation(t16, eps, AF.Exp)
                        g_q[j] = emit_dve_chain(j, eps, t16)

                def emit_e_elem(j):
                    mt, p = pairs[j]
                    msl = slice(mt * MT, (mt + 1) * MT)
                    nA, nB = 2 * p, 2 * p + 1
                    eps = pse.tile([128, 2 * MT], F32, tag="eps", name="eps")
                    nc.tensor.matmul(eps[:, 0:MT],
                                     q_sb[:CQ, nA * 128:(nA + 1) * 128],
                                     k_sb[:CQ, msl], start=True, stop=True)
                    nc.tensor.matmul(eps[:, MT:2 * MT],
                                     q_sb[CQ:2 * CQ, nB * 128:(nB + 1) * 128],
                                     k_sb[CQ:2 * CQ, msl],
                                     start=True, stop=True)
                    g16 = elem.tile([128, 2 * MT], BF16, tag="g", name="g16")
                    # G = min(exp(E),1) + relu(E) = elu(E)+1 exactly.
                    if VARIANT == "v5xnoelem":
                        nc.vector.tensor_copy(g16, eps)
                        return g16
                    if VARIANT == "v5xexp":
                        nc.scalar.activation(g16, eps, AF.Exp)
                        return g16
                    if VARIANT == "v5xexp2":
                        # sane-range exp (perf probe only): e^(0.02E) ~ [.5,2]
                        nc.scalar.activation(g16, eps, AF.Exp, scale=0.02)
                        return g16
                    if V3:
                        # G = elu(E)+1 = min(exp(E),1) + relu(E)
                        #             = min(exp(E), 1 + relu(E))
                        t16 = elem.tile([128, 2 * MT], BF16, tag="t",
                                        name="t16")
                        nc.scalar.activation(t16, eps, AF.Exp)
                        if VARIANT == "v5xdve":
                            act_relu = False
                        elif VARIANT == "v5xact":
                            act_relu = True
                        else:
                            act_relu = (j * ACT_RELU32) % 32 < ACT_RELU32
                        if VARIANT in ("v5b", "v5d", "v7"):
                            # free eps at the FIRST DVE op (shorter PSUM
                            # ring occupancy -> less PE stall on E(j+3))
                            r16 = elem.tile([128, 2 * MT], BF16, tag="r",
                                            name="r16")
                            nc.vector.tensor_scalar(r16, eps, 0.0, 1.0,
                                                    AL.max, AL.add)
                            nc.vector.tensor_tensor(g16, r16, t16, AL.min)
                        elif act_relu:
                            r16 = elem.tile([128, 2 * MT], BF16, tag="r",
                                            name="r16")
                            nc.scalar.activation(r16, eps, AF.Relu)
                            nc.vector.scalar_tensor_tensor(
                                g16, r16, 1.0, t16, AL.add, AL.min)
                        else:
                            # DVE-only: cheap 4x clamp on t16, then one
                            # PSUM-reading STT: G = min(t,1) + relu(E)
                            m16 = elem.tile([128, 2 * MT], BF16, tag="m",
                                            name="m16")
                            nc.vector.tensor_scalar(m16, t16, 1.0, None,
                                                    AL.min)
                            nc.vector.scalar_tensor_tensor(
                                g16, eps, 0.0, m16, AL.max, AL.add)
                        return g16
                    if VARIANT in ("v2", "v2p"):
                        # (broken numerics -- kept for perf A/B only)
                        t16 = elem.tile([128, 2 * MT], BF16, tag="t",
                                        name="t16")
                        nc.scalar.activation(t16, eps, AF.Exp)
                        if VARIANT == "v2p" and j % 16 >= DVE_COMBINE_UPTO:
                            nc.gpsimd.scalar_tensor_tensor(
                                g16, eps, 1.0, t16, AL.add, AL.min)
                        else:
                            nc.vector.scalar_tensor_tensor(
                                g16, eps, 1.0, t16, AL.add, AL.min)
                        return g16
                    if VARIANT == "noelem":
                        nc.vector.tensor_copy(g16, eps)
                        return g16
                    if VARIANT == "actonly":
                        nc.scalar.activation(g16, eps, AF.Exp)
                        return g16
                    t16 = elem.tile([128, 2 * MT], BF16, tag="t", name="t16")
                    a16 = elem.tile([128, 2 * MT], BF16, tag="a", name="a16")
                    if VARIANT in ("cfga", "cfgb"):
                        nc.scalar.activation(t16, eps, AF.Exp)
                        if VARIANT == "cfga" or j % 2 == 0:
                            nc.scalar.activation(a16, eps, AF.Relu)
                        else:
                            nc.vector.tensor_scalar(a16, eps, 0.0, None,
                                                    AL.max)
                        nc.vector.scalar_tensor_tensor(
                            g16, t16, 1.0, a16, AL.min, AL.add)
                        return g16
                    if VARIANT == "dveonly":
                        nc.vector.tensor_scalar(a16, eps, 0.0, None, AL.max)
                        nc.vector.scalar_tensor_tensor(
                            g16, a16, 1.0, a16, AL.min, AL.add)
                        return g16
                    if VARIANT == "poolonly":
                        nc.vector.tensor_copy(t16, eps)
                        nc.gpsimd.tensor_scalar(a16, t16, 1.0, None, AL.min)
                        nc.gpsimd.tensor_tensor(g16, a16, t16, AL.add)
                        return g16
                    nc.scalar.activation(t16, eps, AF.Exp)
                    if j % ACT_RELU_EVERY == 1:
                        nc.scalar.activation(a16, eps, AF.Relu)
                    else:
                        nc.vector.tensor_scalar(a16, eps, 0.0, None, AL.max)
                    if j % 16 < DVE_COMBINE_UPTO:
                        nc.vector.scalar_tensor_tensor(
                            g16, t16, 1.0, a16, AL.min, AL.add)
                    else:
                        u16 = elem.tile([128, 2 * MT], BF16, tag="u",
                                        name="u16")
                        nc.gpsimd.tensor_scalar(u16, t16, 1.0, None, AL.min)
                        nc.gpsimd.tensor_tensor(g16, u16, a16, AL.add)
                    return g16

                def emit_out(i):
                    mt, p = pairs[i]
                    nA, nB = 2 * p, 2 * p + 1
                    g16 = g_q.pop(i)
                    if VARIANT == "nomm" and p > 0:
                        return
                    if p == 0:
                        o_ps[mt] = [pso.tile([128, MT], F32, tag=f"o{ci}",
                                             name=f"o_ps{ci}")
                                    for ci in range(2)]
                    for ci in range(2):
                        csl = slice(ci * 128, (ci + 1) * 128)
                        nc.tensor.matmul(o_ps[mt][ci], vt_sb[:, nA, csl],
                                         g16[:, 0:MT], start=(p == 0),
                                         stop=(VARIANT == "nomm"))
                        nc.tensor.matmul(o_ps[mt][ci], vt_sb[:, nB, csl],
                                         g16[:, MT:2 * MT], start=False,
                                         stop=(VARIANT == "nomm"
                                               or p == NPAIRS - 1))

                def emit_gamma(mt):
                    msl = slice(mt * MT, (mt + 1) * MT)
                    if V5:
                        # o_ps already holds Wg(v/N)@G; just add bias, store
                        for ci in range(2):
                            fo = finp.tile([128, MT], F32, tag=f"fo{ci}",
                                           name="fo")
                            nc.vector.tensor_scalar(fo, o_ps[mt][ci],
                                                    bge_sb[:, ci:ci + 1],
                                                    None, AL.add)
                            nc.sync.dma_start(od[ci, :, msl], fo)
                        del o_ps[mt]
                        return
                    osb = []
                    for ci in range(2):
                        ob = finp.tile([128, MT], BF16, tag=f"ob{ci}",
                                       name=f"ob{ci}")
                        if V3 or VARIANT in ("cfga", "cfgb"):
                            nc.scalar.activation(ob, o_ps[mt][ci], AF.Copy)
                        else:
                            nc.vector.tensor_copy(ob, o_ps[mt][ci])
                        osb.append(ob)
                    del o_ps[mt]
                    for h in range(2):
                        hsl = slice(h * 128, (h + 1) * 128)
                        if V3:
                            gps = pso.tile([128, MT], F32, tag=f"o{h}",
                                           name="gps")
                        else:
                            gps = psg.tile([128, MT], F32, tag="gps",
                                           name="gps")
                        nc.tensor.matmul(gps, wg_sb[:, 0, hsl], osb[0],
                                         start=True, stop=False)
                        nc.tensor.matmul(gps, wg_sb[:, 1, hsl], osb[1],
                                         start=False, stop=True)
                        fo = finp.tile([128, MT], F32, tag="fo", name="fo")
                        if VARIANT in ("cfga", "cfgb"):
                            nc.scalar.activation(fo, gps, AF.Identity,
                                                 bias=bge_sb[:, h:h + 1],
                                                 scale=1.0)
                        else:
                            nc.vector.tensor_scalar(fo, gps,
                                                    bge_sb[:, h:h + 1],
                                                    None, AL.add)
                        nc.sync.dma_start(od[h, :, msl], fo)

                lag = OUT_LAG if V3 else 1
                for j in range(len(pairs) + lag):
                    if j < len(pairs):
                        if V6:
                            emit_e6(j)
                        else:
                            g_q[j] = emit_e_elem(j)
                    if j >= lag:
                        emit_out(j - lag)
                        if pairs[j - lag][1] == NPAIRS - 1:
                            emit_gamma(pairs[j - lag][0])

            if reps == 1:
                body()
            elif V7 and reps >= 2:
                # 2-body unroll: double-buffered q/k/vt let iteration i+1's
                # prologue overlap iteration i's main-loop tail
                with tc.For_i(0, reps // 2, 1):
                    body()
                    body()
                if reps % 2:
                    body()
            else:
                with tc.For_i(0, reps, 1):
                    body()
    nc.compile()
    return nc


_NC_CACHE = {}


def _get_nc(reps=1, variant=None):
    key = (reps, variant or VARIANT)
    if key not in _NC_CACHE:
        _NC_CACHE[key] = build_nc(reps, variant)
    return _NC_CACHE[key]


def _prep_in_maps(inputs):
    x = np.ascontiguousarray(np.asarray(inputs["x"], dtype=np.float32))
    wq = np.asarray(inputs["query_weight"], np.float32)[:, :, 0, 0]
    bq = np.asarray(inputs["query_bias"], np.float32)
    wk = np.asarray(inputs["key_weight"], np.float32)[:, :, 0, 0]
    bk = np.asarray(inputs["key_bias"], np.float32)
    wv = np.asarray(inputs["value_weight"], np.float32)[:, :, 0, 0]
    bv = np.asarray(inputs["value_bias"], np.float32)
    wg = np.asarray(inputs["gamma_weight"], np.float32)[:, :, 0, 0]
    bg = np.asarray(inputs["gamma_bias"], np.float32)

    qdt = ml_dtypes.bfloat16 if VARIANT == "bf16" else np.float32
    wqT = np.ascontiguousarray(wq.T).reshape(2, 128, CQ).astype(qdt)
    wkT = np.ascontiguousarray(wk.T).reshape(2, 128, CQ).astype(qdt)
    shared = {
        "wqT": wqT, "wkT": wkT,
        "bq": np.ascontiguousarray(bq.reshape(CQ, 1)),
        "bk": np.ascontiguousarray(bk.reshape(CQ, 1)),
        "bg": np.ascontiguousarray(bg.reshape(C, 1)),
        "ones": np.ones((1, 128), qdt),
    }
    if VARIANT.startswith(("v5", "v6", "v7")):
        w2 = wg @ wv / N                       # [C, C] fused value+gamma
        b2 = wg @ bv / N
        shared["w2T"] = np.ascontiguousarray(w2.T).reshape(
            2, 128, C).astype(qdt)
        shared["b2"] = b2.reshape(1, C).astype(qdt)
    else:
        shared["wvT"] = np.ascontiguousarray(wv.T / N).reshape(
            2, 128, C).astype(qdt)
        shared["bv"] = (bv / N).reshape(1, C).astype(qdt)
        shared["wgT16"] = np.ascontiguousarray(wg.T).astype(
            ml_dtypes.bfloat16).reshape(2, 128, C)
    return [dict(shared, x=x[b].reshape(2, 128, N).astype(qdt))
            for b in range(B)]


def _run(inputs, trace=False, reps=1, variant=None):
    if variant is not None:
        global VARIANT
        VARIANT = variant
    nc = _get_nc(reps, variant)
    in_maps = _prep_in_maps(inputs)
    res = run_bass_kernel_spmd(nc, in_maps, core_ids=list(range(NCORES)),
                               trace=trace)
    out = np.stack([r["out"].reshape(C, HH, WW) for r in res.results], axis=0)
    return out, res


def kernel(**inputs):
    out, _ = _run(inputs, trace=False)
    return out

